# revision 12
# baseline (speedup 1.0000x reference)
"""Trainium2 Bass kernel for nn_Discriminator (GRU over [64, 1024, 1024]).

v3: fused single-pass design, data-parallel over batch (8 rows/core).

The GRU scan is LDWEIGHTS/issue-bound on the PE (192 weight tiles per
timestep), and the PE's HAM clock gate only reaches 2.4 GHz if the PE
has NO idle windows. So the x-projection GEMM (xg = W_ih@x) is not a
separate phase: its matmul groups are interleaved *between* scan steps
(~12 N=128 matmuls per step), filling the per-step elementwise tail so
the PE stays continuously busy. xr/xz/bhh_n are injected into PSUM via
identity matmuls (PE-side accumulation, no DVE adds on the critical
path). Gate order r -> n -> z lets the n-gate chain (rhn/npre/tanh/d)
overlap the z matmul group; the only serial tail is sig(z), z*d, n+zd.

Layout ("tile-slot"): SBUF tensors are [128 partitions, (j, b)] where
hidden index hid = j*128 + p (j = k-tile 0..7), b = local batch 0..7.
xg scratch DRAM: [block][partition][tp*192 + m*8 + b] with m = gate-
major (r0..7, z0..7, n0..7) so scan-side reads are dense 64-col slices.
"""

import numpy as np
import ml_dtypes

import jax
from jax.sharding import Mesh, PartitionSpec, NamedSharding
from jax.experimental.shard_map import shard_map

import concourse.bass as bass
import concourse.mybir as mybir
import concourse.tile as tile
from concourse import bacc, bass2jax
from concourse.bass import ds

F32 = mybir.dt.float32
BF16 = mybir.dt.bfloat16
AF = mybir.ActivationFunctionType
OP = mybir.AluOpType

B, T, S = 64, 1024, 1024
N_CORES = 8
BC = B // N_CORES      # 8 local batch rows
KT = S // 128          # 8 hidden k-tiles
MT = 3 * KT            # 24 gate m-tiles
JB = KT * BC           # 64 slot-layout free size
G3S = 3 * S
TB = 8                 # timesteps per scan block
NBLK = T // TB         # 128 blocks
FW = 3 * JB            # 192 xg cols per timestep
BLKF = TB * FW         # 1536 xg cols per block
P1N = 2 * TB * BC      # 128 free cols per interleaved x-proj slice (2 blocks)


def _p1_group(nc, ps, wih_sb, xt2, m):
    """One x-projection m-group: 8 accumulating N=128 matmuls."""
    for k in range(KT):
        nc.tensor.matmul(
            ps[:, :],
            wih_sb[:, k * G3S + m * 128 : k * G3S + (m + 1) * 128],
            xt2[:, k * P1N : (k + 1) * P1N],
            start=(k == 0),
            stop=(k == KT - 1),
        )


def _p1_evict(nc, stg, ps, biasm_sb, m):
    stg4 = stg[:, :].rearrange("p (q t m) -> p q t m", q=2, t=TB)
    nc.vector.tensor_scalar_add(
        stg4[:, :, :, ds(m * BC, BC)],
        ps[:, :].rearrange("p (q t b) -> p q t b", q=2, t=TB),
        biasm_sb[:, m : m + 1],
    )


def _build():
    nc = bacc.Bacc("TRN2", target_bir_lowering=False, num_devices=N_CORES)

    XPAD = T * BC + 2 * TB * BC  # +128 cols so blk+2 prefetch stays in bounds
    xT = nc.dram_tensor("xT", [S, XPAD], BF16, kind="ExternalInput")
    wih = nc.dram_tensor("wih", [S, 3 * S], BF16, kind="ExternalInput")
    whh = nc.dram_tensor("whh", [S, 3 * S], BF16, kind="ExternalInput")
    biasm = nc.dram_tensor("biasm", [128, MT], F32, kind="ExternalInput")
    bhhn = nc.dram_tensor("bhhn", [128, JB], BF16, kind="ExternalInput")
    ident = nc.dram_tensor("ident", [128, 128], BF16, kind="ExternalInput")
    hT_out = nc.dram_tensor("hT", [128, JB], F32, kind="ExternalOutput")

    xg2 = nc.dram_tensor("xg_scratch", [NBLK + 2, 128, BLKF], BF16)

    wih_sb = nc.alloc_sbuf_tensor("wih_sb", [128, KT * G3S], BF16)
    whh_sb = nc.alloc_sbuf_tensor("whh_sb", [128, KT * G3S], BF16)
    biasm_sb = nc.alloc_sbuf_tensor("biasm_sb", [128, MT], F32)
    bhhn_sb = nc.alloc_sbuf_tensor("bhhn_sb", [128, JB], BF16)
    ident_sb = nc.alloc_sbuf_tensor("ident_sb", [128, 128], BF16)
    h_a = nc.alloc_sbuf_tensor("h_a", [128, JB], BF16)
    h_b = nc.alloc_sbuf_tensor("h_b", [128, JB], BF16)
    xt2p = nc.alloc_sbuf_tensor("xt2p", [128, KT * P1N], BF16)
    stgp = nc.alloc_sbuf_tensor("stgp", [128, 2 * BLKF], BF16)

    wihR = wih.rearrange("(k p) g -> p k g", p=128)
    whhR = whh.rearrange("(k p) g -> p k g", p=128)
    xTr = xT.rearrange("(k p) f -> p k f", p=128)

    with tile.TileContext(nc) as tc:
        nc.sync.dma_start(out=wih_sb[:, :].rearrange("p (k g) -> p k g", k=KT), in_=wihR)
        nc.sync.dma_start(out=whh_sb[:, :].rearrange("p (k g) -> p k g", k=KT), in_=whhR)
        nc.sync.dma_start(out=biasm_sb[:, :], in_=biasm[:, :])
        nc.sync.dma_start(out=bhhn_sb[:, :], in_=bhhn[:, :])
        nc.sync.dma_start(out=ident_sb[:, :], in_=ident[:, :])
        nc.vector.memset(h_a[:, :], 0.0)

        with tc.tile_pool(name="sc_xg", bufs=2) as xg_pool, \
             tc.tile_pool(name="sc_xt", bufs=2) as xt_pool, \
             tc.tile_pool(name="sc_st", bufs=2) as st_pool, \
             tc.tile_pool(name="sc_ps1", bufs=1, space="PSUM") as sps1_pool, \
             tc.tile_pool(name="sc_ps2", bufs=2, space="PSUM") as sps2_pool, \
             tc.tile_pool(name="p1_ps", bufs=2, space="PSUM") as p1ps_pool, \
             tc.tile_pool(name="sc_ew", bufs=2) as ew_pool:

            # ---- prologue: x-projection for blocks 0,1 ----
            nc.sync.dma_start(
                out=xt2p[:, :].rearrange("p (k f) -> p k f", k=KT),
                in_=xTr[:, :, 0:P1N],
            )
            for m in range(MT):
                ps = p1ps_pool.tile([128, P1N], F32)
                _p1_group(nc, ps, wih_sb, xt2p, m)
                _p1_evict(nc, stgp, ps, biasm_sb, m)
            nc.sync.dma_start(
                out=xg2[0:2, :, :].rearrange("q p f -> p q f"),
                in_=stgp[:, :].rearrange("p (q f) -> p q f", q=2),
            )

            # ---- fused main loop: 2 scan blocks + x-proj for blocks +2,+3 ----
            with tc.For_i(0, NBLK, 2, hint_engines=(mybir.EngineType.PE,)) as blk:
                xt2 = xt_pool.tile([128, KT * P1N], BF16)
                nc.sync.dma_start(
                    out=xt2[:, :].rearrange("p (k f) -> p k f", k=KT),
                    in_=xTr[:, :, ds(blk * TB * BC + P1N, P1N)],
                )
                xgb = xg_pool.tile([128, 2 * BLKF], BF16)
                nc.sync.dma_start(
                    out=xgb[:, :].rearrange("p (q f) -> p q f", q=2),
                    in_=xg2[ds(blk, 2), :, :].rearrange("q p f -> p q f"),
                )
                stg = st_pool.tile([128, 2 * BLKF], BF16)

                for gstep in range(2 * TB):
                    xo = gstep * FW  # xgb col offset for this step
                    h_cur = h_a if gstep % 2 == 0 else h_b
                    h_nxt = h_b if gstep % 2 == 0 else h_a
                    ps_r = sps1_pool.tile([128, JB], F32, tag="ps_r")
                    ps_n = sps1_pool.tile([128, JB], F32, tag="ps_n")
                    ps_z = sps2_pool.tile([128, JB], F32, tag="ps_z")
                    # xr/xz/bhh_n -> PSUM via identity matmuls (no h dep:
                    # these run during the previous step's tail)
                    nc.tensor.matmul(ps_r[:, :], ident_sb[:, :],
                                     xgb[:, ds(xo, JB)], start=True, stop=False)
                    nc.tensor.matmul(ps_z[:, :], ident_sb[:, :],
                                     xgb[:, ds(xo + JB, JB)], start=True, stop=False)
                    nc.tensor.matmul(ps_n[:, :], ident_sb[:, :],
                                     bhhn_sb[:, :], start=True, stop=False)
                    # r gates (m = 0..7)
                    for j in range(KT):
                        for k in range(KT):
                            nc.tensor.matmul(
                                ps_r[:, ds(j * BC, BC)],
                                whh_sb[:, k * G3S + j * 128 : k * G3S + (j + 1) * 128],
                                h_cur[:, ds(k * BC, BC)],
                                start=False, stop=(k == KT - 1),
                            )
                    r_sb = ew_pool.tile([128, JB], BF16, tag="r")
                    nc.scalar.activation(r_sb[:, :], ps_r[:, :], AF.Sigmoid)
                    # n gates (m = 16..23)
                    for j in range(KT):
                        for k in range(KT):
                            m = 2 * KT + j
                            nc.tensor.matmul(
                                ps_n[:, ds(j * BC, BC)],
                                whh_sb[:, k * G3S + m * 128 : k * G3S + (m + 1) * 128],
                                h_cur[:, ds(k * BC, BC)],
                                start=False, stop=(k == KT - 1),
                            )
                    # n chain — pinned early so the scheduler overlaps it
                    # with the z matmul group instead of serializing after
                    with tc.high_priority(offset=100):
                        rhn = ew_pool.tile([128, JB], F32, tag="rhn")
                        nc.vector.tensor_tensor(rhn[:, :], r_sb[:, :], ps_n[:, :], OP.mult)
                        npre = ew_pool.tile([128, JB], F32, tag="npre")
                        nc.vector.tensor_tensor(
                            npre[:, :], rhn[:, :], xgb[:, ds(xo + 2 * JB, JB)], OP.add)
                        n_sb = ew_pool.tile([128, JB], BF16, tag="n")
                        nc.scalar.activation(n_sb[:, :], npre[:, :], AF.Tanh)
                        d_sb = ew_pool.tile([128, JB], BF16, tag="d")
                        nc.vector.tensor_tensor(d_sb[:, :], h_cur[:, :], n_sb[:, :], OP.subtract)
                    # z gates (m = 8..15)
                    for j in range(KT):
                        for k in range(KT):
                            m = KT + j
                            nc.tensor.matmul(
                                ps_z[:, ds(j * BC, BC)],
                                whh_sb[:, k * G3S + m * 128 : k * G3S + (m + 1) * 128],
                                h_cur[:, ds(k * BC, BC)],
                                start=False, stop=(k == KT - 1),
                            )
                    # tail: sig(z), z*d, h' = n + z*d
                    z_sb = ew_pool.tile([128, JB], BF16, tag="z")
                    nc.scalar.activation(z_sb[:, :], ps_z[:, :], AF.Sigmoid)
                    zd = ew_pool.tile([128, JB], BF16, tag="zd")
                    nc.vector.tensor_tensor(zd[:, :], z_sb[:, :], d_sb[:, :], OP.mult)
                    nc.vector.tensor_tensor(h_nxt[:, :], n_sb[:, :], zd[:, :], OP.add)
                    # interleaved x-projection groups (fill this step's tail)
                    m_lo = (gstep * MT) // (2 * TB)
                    m_hi = ((gstep + 1) * MT) // (2 * TB)
                    for m in range(m_lo, m_hi):
                        psp = p1ps_pool.tile([128, P1N], F32)
                        _p1_group(nc, psp, wih_sb, xt2, m)
                        _p1_evict(nc, stg, psp, biasm_sb, m)

                nc.sync.dma_start(
                    out=xg2[ds(blk + 2, 2), :, :].rearrange("q p f -> p q f"),
                    in_=stg[:, :].rearrange("p (q f) -> p q f", q=2),
                )

        hT_sb = nc.alloc_sbuf_tensor("hT_sb", [128, JB], F32)
        nc.vector.tensor_copy(hT_sb[:, :], h_a[:, :])
        nc.sync.dma_start(out=hT_out[:, :], in_=hT_sb[:, :])

    nc.compile()
    return nc


def _prep_inputs(inputs):
    batch = np.asarray(inputs["batch"], np.float32)
    W_ih = np.asarray(inputs["W_ih"], np.float32)
    W_hh = np.asarray(inputs["W_hh"], np.float32)
    b_ih = np.asarray(inputs["b_ih"], np.float32)
    b_hh = np.asarray(inputs["b_hh"], np.float32)

    wihT = np.ascontiguousarray(W_ih.T).astype(ml_dtypes.bfloat16)
    whhT = np.ascontiguousarray(W_hh.T).astype(ml_dtypes.bfloat16)
    bias = b_ih.copy()
    bias[: 2 * S] += b_hh[: 2 * S]
    biasm = np.ascontiguousarray(bias.reshape(MT, 128).T).astype(np.float32)
    bn = b_hh[2 * S :].reshape(KT, 128).T
    bhhn = np.repeat(bn[:, :, None], BC, axis=2).reshape(128, JB)
    bhhn = bhhn.astype(ml_dtypes.bfloat16)
    identm = np.eye(128, dtype=ml_dtypes.bfloat16)

    in_maps = []
    for c in range(N_CORES):
        bs = batch[c * BC : (c + 1) * BC]
        xTc = np.ascontiguousarray(bs.transpose(2, 1, 0).reshape(S, T * BC))
        xTc = np.concatenate(
            [xTc, np.zeros((S, 2 * TB * BC), xTc.dtype)], axis=1)
        in_maps.append({
            "xT": xTc.astype(ml_dtypes.bfloat16),
            "wih": wihT,
            "whh": whhT,
            "biasm": biasm,
            "bhhn": bhhn,
            "ident": identm,
        })
    return in_maps


class _Runner:
    """Compile once, keep the PJRT executable; run per-core in_maps SPMD."""

    def __init__(self, nc, n_cores):
        bass2jax.install_neuronx_cc_hook()
        self.nc, self.n_cores = nc, n_cores
        pname = nc.partition_id_tensor.name if nc.partition_id_tensor else None
        in_names, out_names, out_avals = [], [], []
        for alloc in nc.m.functions[0].allocations:
            if not isinstance(alloc, mybir.MemoryLocationSet):
                continue
            name = alloc.memorylocations[0].name
            if alloc.kind == "ExternalInput":
                if name != pname:
                    in_names.append(name)
            elif alloc.kind == "ExternalOutput":
                out_names.append(name)
                out_avals.append(
                    jax.core.ShapedArray(tuple(alloc.tensor_shape), mybir.dt.np(alloc.dtype))
                )
        self.in_names, self.out_names, self.out_avals = in_names, out_names, out_avals
        n_params = len(in_names)
        all_in = list(in_names) + list(out_names)
        if pname is not None:
            all_in.append(pname)

        def _body(*args):
            operands = list(args)
            if pname is not None:
                operands.append(bass2jax.partition_id_tensor())
            return tuple(
                bass2jax._bass_exec_p.bind(
                    *operands,
                    out_avals=tuple(out_avals),
                    in_names=tuple(all_in),
                    out_names=tuple(out_names),
                    lowering_input_output_aliases=(),
                    sim_require_finite=False,
                    sim_require_nnan=False,
                    nc=nc,
                )
            )

        devices = jax.devices()[:n_cores]
        self.mesh = Mesh(np.asarray(devices), ("core",))
        specs = (PartitionSpec("core"),) * (n_params + len(out_names))
        self.fn = jax.jit(
            shard_map(_body, mesh=self.mesh, in_specs=specs,
                      out_specs=(PartitionSpec("core"),) * len(out_names),
                      check_rep=False),
            keep_unused=True,
        )

    def run(self, in_maps):
        concat = [
            np.concatenate([np.asarray(in_maps[c][n]) for c in range(self.n_cores)], axis=0)
            for n in self.in_names
        ]
        zeros = [
            np.zeros((self.n_cores * a.shape[0], *a.shape[1:]), a.dtype)
            for a in self.out_avals
        ]
        sh = NamedSharding(self.mesh, PartitionSpec("core"))
        dev = [jax.device_put(a, sh) for a in concat + zeros]
        outs = self.fn(*dev)
        jax.block_until_ready(outs)
        return [
            {
                n: np.asarray(outs[i]).reshape(self.n_cores, *self.out_avals[i].shape)[c]
                for i, n in enumerate(self.out_names)
            }
            for c in range(self.n_cores)
        ]


_CACHED = {}


def kernel(**inputs) -> np.ndarray:
    if "runner" not in _CACHED:
        _CACHED["nc"] = _build()
        _CACHED["runner"] = _Runner(_CACHED["nc"], N_CORES)
    runner = _CACHED["runner"]
    in_maps = _prep_inputs(inputs)
    results = runner.run(in_maps)

    W_out = np.asarray(inputs["W_out"], np.float32)
    b_out = np.asarray(inputs["b_out"], np.float32)
    outs = []
    for c in range(N_CORES):
        hT = np.asarray(results[c]["hT"], np.float32)
        h = hT.reshape(128, KT, BC).transpose(2, 1, 0).reshape(BC, S)
        logits = h @ W_out.T + b_out
        outs.append(1.0 / (1.0 + np.exp(-logits[:, 0])))
    return np.concatenate(outs, 0).astype(np.float32)


# revision 13
# speedup vs baseline: 1.0141x; 1.0141x over previous
"""Trainium2 Bass kernel for nn_Discriminator (GRU over [64, 1024, 1024]).

v3: fused single-pass design, data-parallel over batch (8 rows/core).

The GRU scan is LDWEIGHTS/issue-bound on the PE (192 weight tiles per
timestep), and the PE's HAM clock gate only reaches 2.4 GHz if the PE
has NO idle windows. So the x-projection GEMM (xg = W_ih@x) is not a
separate phase: its matmul groups are interleaved *between* scan steps
(~12 N=128 matmuls per step), filling the per-step elementwise tail so
the PE stays continuously busy. xr/xz/bhh_n are injected into PSUM via
identity matmuls (PE-side accumulation, no DVE adds on the critical
path). Gate order r -> n -> z lets the n-gate chain (rhn/npre/tanh/d)
overlap the z matmul group; the only serial tail is sig(z), z*d, n+zd.

Layout ("tile-slot"): SBUF tensors are [128 partitions, (j, b)] where
hidden index hid = j*128 + p (j = k-tile 0..7), b = local batch 0..7.
xg scratch DRAM: [block][partition][tp*192 + m*8 + b] with m = gate-
major (r0..7, z0..7, n0..7) so scan-side reads are dense 64-col slices.
"""

import numpy as np
import ml_dtypes

import jax
from jax.sharding import Mesh, PartitionSpec, NamedSharding
from jax.experimental.shard_map import shard_map

import concourse.bass as bass
import concourse.mybir as mybir
import concourse.tile as tile
from concourse import bacc, bass2jax
from concourse.bass import ds

F32 = mybir.dt.float32
BF16 = mybir.dt.bfloat16
AF = mybir.ActivationFunctionType
OP = mybir.AluOpType

B, T, S = 64, 1024, 1024
N_CORES = 8
BC = B // N_CORES      # 8 local batch rows
KT = S // 128          # 8 hidden k-tiles
MT = 3 * KT            # 24 gate m-tiles
JB = KT * BC           # 64 slot-layout free size
G3S = 3 * S
TB = 8                 # timesteps per scan block
NBLK = T // TB         # 128 blocks
FW = 3 * JB            # 192 xg cols per timestep
BLKF = TB * FW         # 1536 xg cols per block
P1N = 2 * TB * BC      # 128 free cols per interleaved x-proj slice (2 blocks)


def _p1_group(nc, ps, wih_sb, xt2, m):
    """One x-projection m-group: 8 accumulating N=128 matmuls."""
    for k in range(KT):
        nc.tensor.matmul(
            ps[:, :],
            wih_sb[:, k * G3S + m * 128 : k * G3S + (m + 1) * 128],
            xt2[:, k * P1N : (k + 1) * P1N],
            start=(k == 0),
            stop=(k == KT - 1),
        )


def _p1_evict(nc, stg, ps, biasm_sb, m):
    stg4 = stg[:, :].rearrange("p (q t m) -> p q t m", q=2, t=TB)
    nc.vector.tensor_scalar_add(
        stg4[:, :, :, ds(m * BC, BC)],
        ps[:, :].rearrange("p (q t b) -> p q t b", q=2, t=TB),
        biasm_sb[:, m : m + 1],
    )


def _build():
    nc = bacc.Bacc("TRN2", target_bir_lowering=False, num_devices=N_CORES)

    XPAD = T * BC + 2 * TB * BC  # +128 cols so blk+2 prefetch stays in bounds
    xT = nc.dram_tensor("xT", [S, XPAD], BF16, kind="ExternalInput")
    wih = nc.dram_tensor("wih", [S, 3 * S], BF16, kind="ExternalInput")
    whh = nc.dram_tensor("whh", [S, 3 * S], BF16, kind="ExternalInput")
    biasm = nc.dram_tensor("biasm", [128, MT], F32, kind="ExternalInput")
    bhhn = nc.dram_tensor("bhhn", [128, JB], BF16, kind="ExternalInput")
    ident = nc.dram_tensor("ident", [128, 128], BF16, kind="ExternalInput")
    hT_out = nc.dram_tensor("hT", [128, JB], F32, kind="ExternalOutput")

    xg2 = nc.dram_tensor("xg_scratch", [NBLK + 2, 128, BLKF], BF16)

    wih_sb = nc.alloc_sbuf_tensor("wih_sb", [128, KT * G3S], BF16)
    whh_sb = nc.alloc_sbuf_tensor("whh_sb", [128, KT * G3S], BF16)
    biasm_sb = nc.alloc_sbuf_tensor("biasm_sb", [128, MT], F32)
    bhhn_sb = nc.alloc_sbuf_tensor("bhhn_sb", [128, JB], BF16)
    ident_sb = nc.alloc_sbuf_tensor("ident_sb", [128, 128], BF16)
    h_a = nc.alloc_sbuf_tensor("h_a", [128, JB], BF16)
    h_b = nc.alloc_sbuf_tensor("h_b", [128, JB], BF16)
    xt2p = nc.alloc_sbuf_tensor("xt2p", [128, KT * P1N], BF16)
    stgp = nc.alloc_sbuf_tensor("stgp", [128, 2 * BLKF], BF16)

    wihR = wih.rearrange("(k p) g -> p k g", p=128)
    whhR = whh.rearrange("(k p) g -> p k g", p=128)
    xTr = xT.rearrange("(k p) f -> p k f", p=128)

    with tile.TileContext(nc) as tc:
        nc.sync.dma_start(out=wih_sb[:, :].rearrange("p (k g) -> p k g", k=KT), in_=wihR)
        nc.sync.dma_start(out=whh_sb[:, :].rearrange("p (k g) -> p k g", k=KT), in_=whhR)
        nc.sync.dma_start(out=biasm_sb[:, :], in_=biasm[:, :])
        nc.sync.dma_start(out=bhhn_sb[:, :], in_=bhhn[:, :])
        nc.sync.dma_start(out=ident_sb[:, :], in_=ident[:, :])
        nc.vector.memset(h_a[:, :], 0.0)

        with tc.tile_pool(name="sc_xg", bufs=2) as xg_pool, \
             tc.tile_pool(name="sc_xt", bufs=2) as xt_pool, \
             tc.tile_pool(name="sc_st", bufs=2) as st_pool, \
             tc.tile_pool(name="sc_psr", bufs=2, space="PSUM") as spsr_pool, \
             tc.tile_pool(name="sc_psn", bufs=1, space="PSUM") as sps1_pool, \
             tc.tile_pool(name="sc_psz", bufs=3, space="PSUM") as sps2_pool, \
             tc.tile_pool(name="p1_ps", bufs=1, space="PSUM") as p1ps_pool, \
             tc.tile_pool(name="sc_ew", bufs=2) as ew_pool:

            # ---- prologue: x-projection for blocks 0,1 ----
            nc.sync.dma_start(
                out=xt2p[:, :].rearrange("p (k f) -> p k f", k=KT),
                in_=xTr[:, :, 0:P1N],
            )
            for m in range(MT):
                ps = p1ps_pool.tile([128, P1N], F32)
                _p1_group(nc, ps, wih_sb, xt2p, m)
                _p1_evict(nc, stgp, ps, biasm_sb, m)
            nc.sync.dma_start(
                out=xg2[0:2, :, :].rearrange("q p f -> p q f"),
                in_=stgp[:, :].rearrange("p (q f) -> p q f", q=2),
            )

            # ---- fused main loop: 2 scan blocks + x-proj for blocks +2,+3 ----
            with tc.For_i(0, NBLK, 2, hint_engines=(mybir.EngineType.PE,)) as blk:
                xt2 = xt_pool.tile([128, KT * P1N], BF16)
                nc.sync.dma_start(
                    out=xt2[:, :].rearrange("p (k f) -> p k f", k=KT),
                    in_=xTr[:, :, ds(blk * TB * BC + P1N, P1N)],
                )
                xgb = xg_pool.tile([128, 2 * BLKF], BF16)
                nc.sync.dma_start(
                    out=xgb[:, :].rearrange("p (q f) -> p q f", q=2),
                    in_=xg2[ds(blk, 2), :, :].rearrange("q p f -> p q f"),
                )
                stg = st_pool.tile([128, 2 * BLKF], BF16)

                for gstep in range(2 * TB):
                    xo = gstep * FW  # xgb col offset for this step
                    h_cur = h_a if gstep % 2 == 0 else h_b
                    h_nxt = h_b if gstep % 2 == 0 else h_a
                    ps_r = spsr_pool.tile([128, JB], F32, tag="ps_r")
                    ps_n = sps1_pool.tile([128, JB], F32, tag="ps_n")
                    ps_z = sps2_pool.tile([128, JB], F32, tag="ps_z")
                    # xr/xz/bhh_n -> PSUM via identity matmuls (no h dep:
                    # these run during the previous step's tail)
                    nc.tensor.matmul(ps_r[:, :], ident_sb[:, :],
                                     xgb[:, ds(xo, JB)], start=True, stop=False)
                    nc.tensor.matmul(ps_z[:, :], ident_sb[:, :],
                                     xgb[:, ds(xo + JB, JB)], start=True, stop=False)
                    nc.tensor.matmul(ps_n[:, :], ident_sb[:, :],
                                     bhhn_sb[:, :], start=True, stop=False)
                    # r gates (m = 0..7)
                    for j in range(KT):
                        for k in range(KT):
                            nc.tensor.matmul(
                                ps_r[:, ds(j * BC, BC)],
                                whh_sb[:, k * G3S + j * 128 : k * G3S + (j + 1) * 128],
                                h_cur[:, ds(k * BC, BC)],
                                start=False, stop=(k == KT - 1),
                            )
                    r_sb = ew_pool.tile([128, JB], BF16, tag="r")
                    nc.scalar.activation(r_sb[:, :], ps_r[:, :], AF.Sigmoid)
                    # n gates (m = 16..23)
                    for j in range(KT):
                        for k in range(KT):
                            m = 2 * KT + j
                            nc.tensor.matmul(
                                ps_n[:, ds(j * BC, BC)],
                                whh_sb[:, k * G3S + m * 128 : k * G3S + (m + 1) * 128],
                                h_cur[:, ds(k * BC, BC)],
                                start=False, stop=(k == KT - 1),
                            )
                    # n chain — pinned early so the scheduler overlaps it
                    # with the z matmul group instead of serializing after
                    with tc.high_priority(offset=100):
                        rhn = ew_pool.tile([128, JB], F32, tag="rhn")
                        nc.vector.tensor_tensor(rhn[:, :], r_sb[:, :], ps_n[:, :], OP.mult)
                        npre = ew_pool.tile([128, JB], F32, tag="npre")
                        nc.vector.tensor_tensor(
                            npre[:, :], rhn[:, :], xgb[:, ds(xo + 2 * JB, JB)], OP.add)
                        n_sb = ew_pool.tile([128, JB], BF16, tag="n")
                        nc.scalar.activation(n_sb[:, :], npre[:, :], AF.Tanh)
                        d_sb = ew_pool.tile([128, JB], BF16, tag="d")
                        nc.vector.tensor_tensor(d_sb[:, :], h_cur[:, :], n_sb[:, :], OP.subtract)
                    # z gates (m = 8..15)
                    for j in range(KT):
                        for k in range(KT):
                            m = KT + j
                            nc.tensor.matmul(
                                ps_z[:, ds(j * BC, BC)],
                                whh_sb[:, k * G3S + m * 128 : k * G3S + (m + 1) * 128],
                                h_cur[:, ds(k * BC, BC)],
                                start=False, stop=(k == KT - 1),
                            )
                    # tail: sig(z), z*d, h' = n + z*d
                    z_sb = ew_pool.tile([128, JB], BF16, tag="z")
                    nc.scalar.activation(z_sb[:, :], ps_z[:, :], AF.Sigmoid)
                    zd = ew_pool.tile([128, JB], BF16, tag="zd")
                    nc.vector.tensor_tensor(zd[:, :], z_sb[:, :], d_sb[:, :], OP.mult)
                    nc.vector.tensor_tensor(h_nxt[:, :], n_sb[:, :], zd[:, :], OP.add)
                    # interleaved x-projection groups (fill this step's tail)
                    m_lo = (gstep * MT) // (2 * TB)
                    m_hi = ((gstep + 1) * MT) // (2 * TB)
                    for m in range(m_lo, m_hi):
                        psp = p1ps_pool.tile([128, P1N], F32)
                        _p1_group(nc, psp, wih_sb, xt2, m)
                        _p1_evict(nc, stg, psp, biasm_sb, m)

                nc.sync.dma_start(
                    out=xg2[ds(blk + 2, 2), :, :].rearrange("q p f -> p q f"),
                    in_=stg[:, :].rearrange("p (q f) -> p q f", q=2),
                )

        hT_sb = nc.alloc_sbuf_tensor("hT_sb", [128, JB], F32)
        nc.vector.tensor_copy(hT_sb[:, :], h_a[:, :])
        nc.sync.dma_start(out=hT_out[:, :], in_=hT_sb[:, :])

    nc.compile()
    return nc


def _prep_inputs(inputs):
    batch = np.asarray(inputs["batch"], np.float32)
    W_ih = np.asarray(inputs["W_ih"], np.float32)
    W_hh = np.asarray(inputs["W_hh"], np.float32)
    b_ih = np.asarray(inputs["b_ih"], np.float32)
    b_hh = np.asarray(inputs["b_hh"], np.float32)

    wihT = np.ascontiguousarray(W_ih.T).astype(ml_dtypes.bfloat16)
    whhT = np.ascontiguousarray(W_hh.T).astype(ml_dtypes.bfloat16)
    bias = b_ih.copy()
    bias[: 2 * S] += b_hh[: 2 * S]
    biasm = np.ascontiguousarray(bias.reshape(MT, 128).T).astype(np.float32)
    bn = b_hh[2 * S :].reshape(KT, 128).T
    bhhn = np.repeat(bn[:, :, None], BC, axis=2).reshape(128, JB)
    bhhn = bhhn.astype(ml_dtypes.bfloat16)
    identm = np.eye(128, dtype=ml_dtypes.bfloat16)

    in_maps = []
    for c in range(N_CORES):
        bs = batch[c * BC : (c + 1) * BC]
        xTc = np.ascontiguousarray(bs.transpose(2, 1, 0).reshape(S, T * BC))
        xTc = np.concatenate(
            [xTc, np.zeros((S, 2 * TB * BC), xTc.dtype)], axis=1)
        in_maps.append({
            "xT": xTc.astype(ml_dtypes.bfloat16),
            "wih": wihT,
            "whh": whhT,
            "biasm": biasm,
            "bhhn": bhhn,
            "ident": identm,
        })
    return in_maps


class _Runner:
    """Compile once, keep the PJRT executable; run per-core in_maps SPMD."""

    def __init__(self, nc, n_cores):
        bass2jax.install_neuronx_cc_hook()
        self.nc, self.n_cores = nc, n_cores
        pname = nc.partition_id_tensor.name if nc.partition_id_tensor else None
        in_names, out_names, out_avals = [], [], []
        for alloc in nc.m.functions[0].allocations:
            if not isinstance(alloc, mybir.MemoryLocationSet):
                continue
            name = alloc.memorylocations[0].name
            if alloc.kind == "ExternalInput":
                if name != pname:
                    in_names.append(name)
            elif alloc.kind == "ExternalOutput":
                out_names.append(name)
                out_avals.append(
                    jax.core.ShapedArray(tuple(alloc.tensor_shape), mybir.dt.np(alloc.dtype))
                )
        self.in_names, self.out_names, self.out_avals = in_names, out_names, out_avals
        n_params = len(in_names)
        all_in = list(in_names) + list(out_names)
        if pname is not None:
            all_in.append(pname)

        def _body(*args):
            operands = list(args)
            if pname is not None:
                operands.append(bass2jax.partition_id_tensor())
            return tuple(
                bass2jax._bass_exec_p.bind(
                    *operands,
                    out_avals=tuple(out_avals),
                    in_names=tuple(all_in),
                    out_names=tuple(out_names),
                    lowering_input_output_aliases=(),
                    sim_require_finite=False,
                    sim_require_nnan=False,
                    nc=nc,
                )
            )

        devices = jax.devices()[:n_cores]
        self.mesh = Mesh(np.asarray(devices), ("core",))
        specs = (PartitionSpec("core"),) * (n_params + len(out_names))
        self.fn = jax.jit(
            shard_map(_body, mesh=self.mesh, in_specs=specs,
                      out_specs=(PartitionSpec("core"),) * len(out_names),
                      check_rep=False),
            keep_unused=True,
        )

    def run(self, in_maps):
        concat = [
            np.concatenate([np.asarray(in_maps[c][n]) for c in range(self.n_cores)], axis=0)
            for n in self.in_names
        ]
        zeros = [
            np.zeros((self.n_cores * a.shape[0], *a.shape[1:]), a.dtype)
            for a in self.out_avals
        ]
        sh = NamedSharding(self.mesh, PartitionSpec("core"))
        dev = [jax.device_put(a, sh) for a in concat + zeros]
        outs = self.fn(*dev)
        jax.block_until_ready(outs)
        return [
            {
                n: np.asarray(outs[i]).reshape(self.n_cores, *self.out_avals[i].shape)[c]
                for i, n in enumerate(self.out_names)
            }
            for c in range(self.n_cores)
        ]


_CACHED = {}


def kernel(**inputs) -> np.ndarray:
    if "runner" not in _CACHED:
        _CACHED["nc"] = _build()
        _CACHED["runner"] = _Runner(_CACHED["nc"], N_CORES)
    runner = _CACHED["runner"]
    in_maps = _prep_inputs(inputs)
    results = runner.run(in_maps)

    W_out = np.asarray(inputs["W_out"], np.float32)
    b_out = np.asarray(inputs["b_out"], np.float32)
    outs = []
    for c in range(N_CORES):
        hT = np.asarray(results[c]["hT"], np.float32)
        h = hT.reshape(128, KT, BC).transpose(2, 1, 0).reshape(BC, S)
        logits = h @ W_out.T + b_out
        outs.append(1.0 / (1.0 + np.exp(-logits[:, 0])))
    return np.concatenate(outs, 0).astype(np.float32)


# revision 14
# speedup vs baseline: 1.0145x; 1.0003x over previous
"""Trainium2 Bass kernel for nn_Discriminator (GRU over [64, 1024, 1024]).

v3: fused single-pass design, data-parallel over batch (8 rows/core).

The GRU scan is LDWEIGHTS/issue-bound on the PE (192 weight tiles per
timestep), and the PE's HAM clock gate only reaches 2.4 GHz if the PE
has NO idle windows. So the x-projection GEMM (xg = W_ih@x) is not a
separate phase: its matmul groups are interleaved *between* scan steps
(~12 N=128 matmuls per step), filling the per-step elementwise tail so
the PE stays continuously busy. xr/xz/bhh_n are injected into PSUM via
identity matmuls (PE-side accumulation, no DVE adds on the critical
path). Gate order r -> n -> z lets the n-gate chain (rhn/npre/tanh/d)
overlap the z matmul group; the only serial tail is sig(z), z*d, n+zd.

Layout ("tile-slot"): SBUF tensors are [128 partitions, (j, b)] where
hidden index hid = j*128 + p (j = k-tile 0..7), b = local batch 0..7.
xg scratch DRAM: [block][partition][tp*192 + m*8 + b] with m = gate-
major (r0..7, z0..7, n0..7) so scan-side reads are dense 64-col slices.
"""

import numpy as np
import ml_dtypes

import jax
from jax.sharding import Mesh, PartitionSpec, NamedSharding
from jax.experimental.shard_map import shard_map

import concourse.bass as bass
import concourse.mybir as mybir
import concourse.tile as tile
from concourse import bacc, bass2jax
from concourse.bass import ds

F32 = mybir.dt.float32
BF16 = mybir.dt.bfloat16
AF = mybir.ActivationFunctionType
OP = mybir.AluOpType

B, T, S = 64, 1024, 1024
N_CORES = 8
BC = B // N_CORES      # 8 local batch rows
KT = S // 128          # 8 hidden k-tiles
MT = 3 * KT            # 24 gate m-tiles
JB = KT * BC           # 64 slot-layout free size
G3S = 3 * S
TB = 8                 # timesteps per scan block
NBLK = T // TB         # 128 blocks
FW = 3 * JB            # 192 xg cols per timestep
BLKF = TB * FW         # 1536 xg cols per block
P1N = 2 * TB * BC      # 128 free cols per interleaved x-proj slice (2 blocks)


def _p1_group(nc, ps, wih_sb, xt2, m):
    """One x-projection m-group: 8 accumulating N=128 matmuls."""
    for k in range(KT):
        nc.tensor.matmul(
            ps[:, :],
            wih_sb[:, k * G3S + m * 128 : k * G3S + (m + 1) * 128],
            xt2[:, k * P1N : (k + 1) * P1N],
            start=(k == 0),
            stop=(k == KT - 1),
        )


def _p1_evict(nc, stg, ps, biasm_sb, m):
    stg4 = stg[:, :].rearrange("p (q t m) -> p q t m", q=2, t=TB)
    nc.vector.tensor_scalar_add(
        stg4[:, :, :, ds(m * BC, BC)],
        ps[:, :].rearrange("p (q t b) -> p q t b", q=2, t=TB),
        biasm_sb[:, m : m + 1],
    )


def _build():
    nc = bacc.Bacc("TRN2", target_bir_lowering=False, num_devices=N_CORES)

    XPAD = T * BC + 2 * TB * BC  # +128 cols so blk+2 prefetch stays in bounds
    xT = nc.dram_tensor("xT", [S, XPAD], BF16, kind="ExternalInput")
    wih = nc.dram_tensor("wih", [S, 3 * S], BF16, kind="ExternalInput")
    whh = nc.dram_tensor("whh", [S, 3 * S], BF16, kind="ExternalInput")
    biasm = nc.dram_tensor("biasm", [128, MT], F32, kind="ExternalInput")
    bhhn = nc.dram_tensor("bhhn", [128, JB], BF16, kind="ExternalInput")
    ident = nc.dram_tensor("ident", [128, 128], BF16, kind="ExternalInput")
    hT_out = nc.dram_tensor("hT", [128, JB], F32, kind="ExternalOutput")

    xg2 = nc.dram_tensor("xg_scratch", [NBLK + 2, 128, BLKF], BF16)

    wih_sb = nc.alloc_sbuf_tensor("wih_sb", [128, KT * G3S], BF16)
    whh_sb = nc.alloc_sbuf_tensor("whh_sb", [128, KT * G3S], BF16)
    biasm_sb = nc.alloc_sbuf_tensor("biasm_sb", [128, MT], F32)
    bhhn_sb = nc.alloc_sbuf_tensor("bhhn_sb", [128, JB], BF16)
    ident_sb = nc.alloc_sbuf_tensor("ident_sb", [128, 128], BF16)
    h_a = nc.alloc_sbuf_tensor("h_a", [128, JB], BF16)
    h_b = nc.alloc_sbuf_tensor("h_b", [128, JB], BF16)
    xt2p = nc.alloc_sbuf_tensor("xt2p", [128, KT * P1N], BF16)
    stgp = nc.alloc_sbuf_tensor("stgp", [128, 2 * BLKF], BF16)

    wihR = wih.rearrange("(k p) g -> p k g", p=128)
    whhR = whh.rearrange("(k p) g -> p k g", p=128)
    xTr = xT.rearrange("(k p) f -> p k f", p=128)

    with tile.TileContext(nc) as tc:
        nc.sync.dma_start(out=wih_sb[:, :].rearrange("p (k g) -> p k g", k=KT), in_=wihR)
        nc.sync.dma_start(out=whh_sb[:, :].rearrange("p (k g) -> p k g", k=KT), in_=whhR)
        nc.sync.dma_start(out=biasm_sb[:, :], in_=biasm[:, :])
        nc.sync.dma_start(out=bhhn_sb[:, :], in_=bhhn[:, :])
        nc.sync.dma_start(out=ident_sb[:, :], in_=ident[:, :])
        nc.vector.memset(h_a[:, :], 0.0)

        with tc.tile_pool(name="sc_xg", bufs=2) as xg_pool, \
             tc.tile_pool(name="sc_xt", bufs=2) as xt_pool, \
             tc.tile_pool(name="sc_st", bufs=2) as st_pool, \
             tc.tile_pool(name="sc_psr", bufs=2, space="PSUM") as spsr_pool, \
             tc.tile_pool(name="sc_psn", bufs=1, space="PSUM") as sps1_pool, \
             tc.tile_pool(name="sc_psz", bufs=3, space="PSUM") as sps2_pool, \
             tc.tile_pool(name="p1_ps", bufs=1, space="PSUM") as p1ps_pool, \
             tc.tile_pool(name="sc_ew", bufs=4) as ew_pool:

            # ---- prologue: x-projection for blocks 0,1 ----
            nc.sync.dma_start(
                out=xt2p[:, :].rearrange("p (k f) -> p k f", k=KT),
                in_=xTr[:, :, 0:P1N],
            )
            for m in range(MT):
                ps = p1ps_pool.tile([128, P1N], F32)
                _p1_group(nc, ps, wih_sb, xt2p, m)
                _p1_evict(nc, stgp, ps, biasm_sb, m)
            nc.sync.dma_start(
                out=xg2[0:2, :, :].rearrange("q p f -> p q f"),
                in_=stgp[:, :].rearrange("p (q f) -> p q f", q=2),
            )

            # ---- fused main loop: 2 scan blocks + x-proj for blocks +2,+3 ----
            with tc.For_i(0, NBLK, 2, hint_engines=(mybir.EngineType.PE,)) as blk:
                xt2 = xt_pool.tile([128, KT * P1N], BF16)
                nc.sync.dma_start(
                    out=xt2[:, :].rearrange("p (k f) -> p k f", k=KT),
                    in_=xTr[:, :, ds(blk * TB * BC + P1N, P1N)],
                )
                xgb = xg_pool.tile([128, 2 * BLKF], BF16)
                nc.sync.dma_start(
                    out=xgb[:, :].rearrange("p (q f) -> p q f", q=2),
                    in_=xg2[ds(blk, 2), :, :].rearrange("q p f -> p q f"),
                )
                stg = st_pool.tile([128, 2 * BLKF], BF16)

                for gstep in range(2 * TB):
                    xo = gstep * FW  # xgb col offset for this step
                    h_cur = h_a if gstep % 2 == 0 else h_b
                    h_nxt = h_b if gstep % 2 == 0 else h_a
                    ps_r = spsr_pool.tile([128, JB], F32, tag="ps_r")
                    ps_n = sps1_pool.tile([128, JB], F32, tag="ps_n")
                    ps_z = sps2_pool.tile([128, JB], F32, tag="ps_z")
                    # xr/xz/bhh_n -> PSUM via identity matmuls (no h dep:
                    # these run during the previous step's tail)
                    nc.tensor.matmul(ps_r[:, :], ident_sb[:, :],
                                     xgb[:, ds(xo, JB)], start=True, stop=False)
                    nc.tensor.matmul(ps_z[:, :], ident_sb[:, :],
                                     xgb[:, ds(xo + JB, JB)], start=True, stop=False)
                    nc.tensor.matmul(ps_n[:, :], ident_sb[:, :],
                                     bhhn_sb[:, :], start=True, stop=False)
                    # r gates (m = 0..7)
                    for j in range(KT):
                        for k in range(KT):
                            nc.tensor.matmul(
                                ps_r[:, ds(j * BC, BC)],
                                whh_sb[:, k * G3S + j * 128 : k * G3S + (j + 1) * 128],
                                h_cur[:, ds(k * BC, BC)],
                                start=False, stop=(k == KT - 1),
                            )
                    r_sb = ew_pool.tile([128, JB], BF16, tag="r")
                    nc.scalar.activation(r_sb[:, :], ps_r[:, :], AF.Sigmoid)
                    # n gates (m = 16..23)
                    for j in range(KT):
                        for k in range(KT):
                            m = 2 * KT + j
                            nc.tensor.matmul(
                                ps_n[:, ds(j * BC, BC)],
                                whh_sb[:, k * G3S + m * 128 : k * G3S + (m + 1) * 128],
                                h_cur[:, ds(k * BC, BC)],
                                start=False, stop=(k == KT - 1),
                            )
                    # n chain — pinned early so the scheduler overlaps it
                    # with the z matmul group instead of serializing after
                    with tc.high_priority(offset=100):
                        rhn = ew_pool.tile([128, JB], F32, tag="rhn")
                        nc.vector.tensor_tensor(rhn[:, :], r_sb[:, :], ps_n[:, :], OP.mult)
                        npre = ew_pool.tile([128, JB], F32, tag="npre")
                        nc.vector.tensor_tensor(
                            npre[:, :], rhn[:, :], xgb[:, ds(xo + 2 * JB, JB)], OP.add)
                        n_sb = ew_pool.tile([128, JB], BF16, tag="n")
                        nc.scalar.activation(n_sb[:, :], npre[:, :], AF.Tanh)
                        d_sb = ew_pool.tile([128, JB], BF16, tag="d")
                        nc.vector.tensor_tensor(d_sb[:, :], h_cur[:, :], n_sb[:, :], OP.subtract)
                    # z gates (m = 8..15)
                    for j in range(KT):
                        for k in range(KT):
                            m = KT + j
                            nc.tensor.matmul(
                                ps_z[:, ds(j * BC, BC)],
                                whh_sb[:, k * G3S + m * 128 : k * G3S + (m + 1) * 128],
                                h_cur[:, ds(k * BC, BC)],
                                start=False, stop=(k == KT - 1),
                            )
                    # tail: sig(z), z*d, h' = n + z*d
                    z_sb = ew_pool.tile([128, JB], BF16, tag="z")
                    nc.scalar.activation(z_sb[:, :], ps_z[:, :], AF.Sigmoid)
                    zd = ew_pool.tile([128, JB], BF16, tag="zd")
                    nc.vector.tensor_tensor(zd[:, :], z_sb[:, :], d_sb[:, :], OP.mult)
                    nc.vector.tensor_tensor(h_nxt[:, :], n_sb[:, :], zd[:, :], OP.add)
                    # interleaved x-projection groups (fill this step's tail)
                    m_lo = (gstep * MT) // (2 * TB)
                    m_hi = ((gstep + 1) * MT) // (2 * TB)
                    for m in range(m_lo, m_hi):
                        psp = p1ps_pool.tile([128, P1N], F32)
                        _p1_group(nc, psp, wih_sb, xt2, m)
                        _p1_evict(nc, stg, psp, biasm_sb, m)

                nc.sync.dma_start(
                    out=xg2[ds(blk + 2, 2), :, :].rearrange("q p f -> p q f"),
                    in_=stg[:, :].rearrange("p (q f) -> p q f", q=2),
                )

        hT_sb = nc.alloc_sbuf_tensor("hT_sb", [128, JB], F32)
        nc.vector.tensor_copy(hT_sb[:, :], h_a[:, :])
        nc.sync.dma_start(out=hT_out[:, :], in_=hT_sb[:, :])

    nc.compile()
    return nc


def _prep_inputs(inputs):
    batch = np.asarray(inputs["batch"], np.float32)
    W_ih = np.asarray(inputs["W_ih"], np.float32)
    W_hh = np.asarray(inputs["W_hh"], np.float32)
    b_ih = np.asarray(inputs["b_ih"], np.float32)
    b_hh = np.asarray(inputs["b_hh"], np.float32)

    wihT = np.ascontiguousarray(W_ih.T).astype(ml_dtypes.bfloat16)
    whhT = np.ascontiguousarray(W_hh.T).astype(ml_dtypes.bfloat16)
    bias = b_ih.copy()
    bias[: 2 * S] += b_hh[: 2 * S]
    biasm = np.ascontiguousarray(bias.reshape(MT, 128).T).astype(np.float32)
    bn = b_hh[2 * S :].reshape(KT, 128).T
    bhhn = np.repeat(bn[:, :, None], BC, axis=2).reshape(128, JB)
    bhhn = bhhn.astype(ml_dtypes.bfloat16)
    identm = np.eye(128, dtype=ml_dtypes.bfloat16)

    in_maps = []
    for c in range(N_CORES):
        bs = batch[c * BC : (c + 1) * BC]
        xTc = np.ascontiguousarray(bs.transpose(2, 1, 0).reshape(S, T * BC))
        xTc = np.concatenate(
            [xTc, np.zeros((S, 2 * TB * BC), xTc.dtype)], axis=1)
        in_maps.append({
            "xT": xTc.astype(ml_dtypes.bfloat16),
            "wih": wihT,
            "whh": whhT,
            "biasm": biasm,
            "bhhn": bhhn,
            "ident": identm,
        })
    return in_maps


class _Runner:
    """Compile once, keep the PJRT executable; run per-core in_maps SPMD."""

    def __init__(self, nc, n_cores):
        bass2jax.install_neuronx_cc_hook()
        self.nc, self.n_cores = nc, n_cores
        pname = nc.partition_id_tensor.name if nc.partition_id_tensor else None
        in_names, out_names, out_avals = [], [], []
        for alloc in nc.m.functions[0].allocations:
            if not isinstance(alloc, mybir.MemoryLocationSet):
                continue
            name = alloc.memorylocations[0].name
            if alloc.kind == "ExternalInput":
                if name != pname:
                    in_names.append(name)
            elif alloc.kind == "ExternalOutput":
                out_names.append(name)
                out_avals.append(
                    jax.core.ShapedArray(tuple(alloc.tensor_shape), mybir.dt.np(alloc.dtype))
                )
        self.in_names, self.out_names, self.out_avals = in_names, out_names, out_avals
        n_params = len(in_names)
        all_in = list(in_names) + list(out_names)
        if pname is not None:
            all_in.append(pname)

        def _body(*args):
            operands = list(args)
            if pname is not None:
                operands.append(bass2jax.partition_id_tensor())
            return tuple(
                bass2jax._bass_exec_p.bind(
                    *operands,
                    out_avals=tuple(out_avals),
                    in_names=tuple(all_in),
                    out_names=tuple(out_names),
                    lowering_input_output_aliases=(),
                    sim_require_finite=False,
                    sim_require_nnan=False,
                    nc=nc,
                )
            )

        devices = jax.devices()[:n_cores]
        self.mesh = Mesh(np.asarray(devices), ("core",))
        specs = (PartitionSpec("core"),) * (n_params + len(out_names))
        self.fn = jax.jit(
            shard_map(_body, mesh=self.mesh, in_specs=specs,
                      out_specs=(PartitionSpec("core"),) * len(out_names),
                      check_rep=False),
            keep_unused=True,
        )

    def run(self, in_maps):
        concat = [
            np.concatenate([np.asarray(in_maps[c][n]) for c in range(self.n_cores)], axis=0)
            for n in self.in_names
        ]
        zeros = [
            np.zeros((self.n_cores * a.shape[0], *a.shape[1:]), a.dtype)
            for a in self.out_avals
        ]
        sh = NamedSharding(self.mesh, PartitionSpec("core"))
        dev = [jax.device_put(a, sh) for a in concat + zeros]
        outs = self.fn(*dev)
        jax.block_until_ready(outs)
        return [
            {
                n: np.asarray(outs[i]).reshape(self.n_cores, *self.out_avals[i].shape)[c]
                for i, n in enumerate(self.out_names)
            }
            for c in range(self.n_cores)
        ]


_CACHED = {}


def kernel(**inputs) -> np.ndarray:
    if "runner" not in _CACHED:
        _CACHED["nc"] = _build()
        _CACHED["runner"] = _Runner(_CACHED["nc"], N_CORES)
    runner = _CACHED["runner"]
    in_maps = _prep_inputs(inputs)
    results = runner.run(in_maps)

    W_out = np.asarray(inputs["W_out"], np.float32)
    b_out = np.asarray(inputs["b_out"], np.float32)
    outs = []
    for c in range(N_CORES):
        hT = np.asarray(results[c]["hT"], np.float32)
        h = hT.reshape(128, KT, BC).transpose(2, 1, 0).reshape(BC, S)
        logits = h @ W_out.T + b_out
        outs.append(1.0 / (1.0 + np.exp(-logits[:, 0])))
    return np.concatenate(outs, 0).astype(np.float32)


# revision 15
# speedup vs baseline: 1.0151x; 1.0007x over previous
"""Trainium2 Bass kernel for nn_Discriminator (GRU over [64, 1024, 1024]).

v3: fused single-pass design, data-parallel over batch (8 rows/core).

The GRU scan is LDWEIGHTS/issue-bound on the PE (192 weight tiles per
timestep), and the PE's HAM clock gate only reaches 2.4 GHz if the PE
has NO idle windows. So the x-projection GEMM (xg = W_ih@x) is not a
separate phase: its matmul groups are interleaved *between* scan steps
(~12 N=128 matmuls per step), filling the per-step elementwise tail so
the PE stays continuously busy. xr/xz/bhh_n are injected into PSUM via
identity matmuls (PE-side accumulation, no DVE adds on the critical
path). Gate order r -> n -> z lets the n-gate chain (rhn/npre/tanh/d)
overlap the z matmul group; the only serial tail is sig(z), z*d, n+zd.

Layout ("tile-slot"): SBUF tensors are [128 partitions, (j, b)] where
hidden index hid = j*128 + p (j = k-tile 0..7), b = local batch 0..7.
xg scratch DRAM: [block][partition][tp*192 + m*8 + b] with m = gate-
major (r0..7, z0..7, n0..7) so scan-side reads are dense 64-col slices.
"""

import numpy as np
import ml_dtypes

import jax
from jax.sharding import Mesh, PartitionSpec, NamedSharding
from jax.experimental.shard_map import shard_map

import concourse.bass as bass
import concourse.mybir as mybir
import concourse.tile as tile
from concourse import bacc, bass2jax
from concourse.bass import ds

F32 = mybir.dt.float32
BF16 = mybir.dt.bfloat16
AF = mybir.ActivationFunctionType
OP = mybir.AluOpType

B, T, S = 64, 1024, 1024
N_CORES = 8
BC = B // N_CORES      # 8 local batch rows
KT = S // 128          # 8 hidden k-tiles
MT = 3 * KT            # 24 gate m-tiles
JB = KT * BC           # 64 slot-layout free size
G3S = 3 * S
TB = 8                 # timesteps per scan block
NBLK = T // TB         # 128 blocks
FW = 3 * JB            # 192 xg cols per timestep
BLKF = TB * FW         # 1536 xg cols per block
P1N = 2 * TB * BC      # 128 free cols per interleaved x-proj slice (2 blocks)


def _p1_group(nc, ps, wih_sb, xt2, m):
    """One x-projection m-group: 8 accumulating N=128 matmuls."""
    for k in range(KT):
        nc.tensor.matmul(
            ps[:, :],
            wih_sb[:, k * G3S + m * 128 : k * G3S + (m + 1) * 128],
            xt2[:, k * P1N : (k + 1) * P1N],
            start=(k == 0),
            stop=(k == KT - 1),
        )


def _p1_evict(nc, stg, ps, biasm_sb, m):
    stg4 = stg[:, :].rearrange("p (q t m) -> p q t m", q=2, t=TB)
    nc.vector.tensor_scalar_add(
        stg4[:, :, :, ds(m * BC, BC)],
        ps[:, :].rearrange("p (q t b) -> p q t b", q=2, t=TB),
        biasm_sb[:, m : m + 1],
    )


def _build():
    nc = bacc.Bacc("TRN2", target_bir_lowering=False, num_devices=N_CORES)

    XPAD = T * BC + 2 * TB * BC  # +128 cols so blk+2 prefetch stays in bounds
    xT = nc.dram_tensor("xT", [S, XPAD], BF16, kind="ExternalInput")
    wih = nc.dram_tensor("wih", [S, 3 * S], BF16, kind="ExternalInput")
    whh = nc.dram_tensor("whh", [S, 3 * S], BF16, kind="ExternalInput")
    biasm = nc.dram_tensor("biasm", [128, MT], F32, kind="ExternalInput")
    bhhn = nc.dram_tensor("bhhn", [128, JB], BF16, kind="ExternalInput")
    ident = nc.dram_tensor("ident", [128, 128], BF16, kind="ExternalInput")
    hT_out = nc.dram_tensor("hT", [128, JB], F32, kind="ExternalOutput")

    xg2 = nc.dram_tensor("xg_scratch", [NBLK + 2, 128, BLKF], BF16)

    wih_sb = nc.alloc_sbuf_tensor("wih_sb", [128, KT * G3S], BF16)
    whh_sb = nc.alloc_sbuf_tensor("whh_sb", [128, KT * G3S], BF16)
    biasm_sb = nc.alloc_sbuf_tensor("biasm_sb", [128, MT], F32)
    bhhn_sb = nc.alloc_sbuf_tensor("bhhn_sb", [128, JB], BF16)
    ident_sb = nc.alloc_sbuf_tensor("ident_sb", [128, 128], BF16)
    h_a = nc.alloc_sbuf_tensor("h_a", [128, JB], BF16)
    h_b = nc.alloc_sbuf_tensor("h_b", [128, JB], BF16)
    xt2p = nc.alloc_sbuf_tensor("xt2p", [128, KT * P1N], BF16)
    stgp = nc.alloc_sbuf_tensor("stgp", [128, 2 * BLKF], BF16)

    wihR = wih.rearrange("(k p) g -> p k g", p=128)
    whhR = whh.rearrange("(k p) g -> p k g", p=128)
    xTr = xT.rearrange("(k p) f -> p k f", p=128)

    with tile.TileContext(nc) as tc:
        nc.sync.dma_start(out=wih_sb[:, :].rearrange("p (k g) -> p k g", k=KT), in_=wihR)
        nc.sync.dma_start(out=whh_sb[:, :].rearrange("p (k g) -> p k g", k=KT), in_=whhR)
        nc.sync.dma_start(out=biasm_sb[:, :], in_=biasm[:, :])
        nc.sync.dma_start(out=bhhn_sb[:, :], in_=bhhn[:, :])
        nc.sync.dma_start(out=ident_sb[:, :], in_=ident[:, :])
        nc.vector.memset(h_a[:, :], 0.0)

        with tc.tile_pool(name="sc_xg", bufs=2) as xg_pool, \
             tc.tile_pool(name="sc_xt", bufs=2) as xt_pool, \
             tc.tile_pool(name="sc_st", bufs=2) as st_pool, \
             tc.tile_pool(name="sc_psr", bufs=2, space="PSUM") as spsr_pool, \
             tc.tile_pool(name="sc_psn", bufs=1, space="PSUM") as sps1_pool, \
             tc.tile_pool(name="sc_psz", bufs=3, space="PSUM") as sps2_pool, \
             tc.tile_pool(name="p1_ps", bufs=1, space="PSUM") as p1ps_pool, \
             tc.tile_pool(name="sc_ew", bufs=4) as ew_pool:

            # ---- prologue: x-projection for blocks 0,1 ----
            nc.sync.dma_start(
                out=xt2p[:, :].rearrange("p (k f) -> p k f", k=KT),
                in_=xTr[:, :, 0:P1N],
            )
            for m in range(MT):
                ps = p1ps_pool.tile([128, P1N], F32)
                _p1_group(nc, ps, wih_sb, xt2p, m)
                _p1_evict(nc, stgp, ps, biasm_sb, m)
            nc.sync.dma_start(
                out=xg2[0:2, :, :].rearrange("q p f -> p q f"),
                in_=stgp[:, :].rearrange("p (q f) -> p q f", q=2),
            )

            # ---- fused main loop: 2 scan blocks + x-proj for blocks +2,+3 ----
            with tc.For_i(0, NBLK, 2, hint_engines=(mybir.EngineType.PE,)) as blk:
                xt2 = xt_pool.tile([128, KT * P1N], BF16)
                nc.sync.dma_start(
                    out=xt2[:, :].rearrange("p (k f) -> p k f", k=KT),
                    in_=xTr[:, :, ds(blk * TB * BC + P1N, P1N)],
                )
                xgb = xg_pool.tile([128, 2 * BLKF], BF16)
                nc.sync.dma_start(
                    out=xgb[:, :].rearrange("p (q f) -> p q f", q=2),
                    in_=xg2[ds(blk, 2), :, :].rearrange("q p f -> p q f"),
                )
                stg = st_pool.tile([128, 2 * BLKF], BF16)

                for gstep in range(2 * TB):
                    xo = gstep * FW  # xgb col offset for this step
                    h_cur = h_a if gstep % 2 == 0 else h_b
                    h_nxt = h_b if gstep % 2 == 0 else h_a
                    ps_r = spsr_pool.tile([128, JB], F32, tag="ps_r")
                    ps_n = sps1_pool.tile([128, JB], F32, tag="ps_n")
                    ps_z = sps2_pool.tile([128, JB], F32, tag="ps_z")
                    # xr/xz/bhh_n -> PSUM via identity matmuls (no h dep:
                    # these run during the previous step's tail)
                    nc.tensor.matmul(ps_n[:, :], ident_sb[:, :],
                                     bhhn_sb[:, :], start=True, stop=False)
                    nc.tensor.matmul(ps_r[:, :], ident_sb[:, :],
                                     xgb[:, ds(xo, JB)], start=True, stop=False)
                    nc.tensor.matmul(ps_z[:, :], ident_sb[:, :],
                                     xgb[:, ds(xo + JB, JB)], start=True, stop=False)
                    # r gates (m = 0..7)
                    for j in range(KT):
                        for k in range(KT):
                            nc.tensor.matmul(
                                ps_r[:, ds(j * BC, BC)],
                                whh_sb[:, k * G3S + j * 128 : k * G3S + (j + 1) * 128],
                                h_cur[:, ds(k * BC, BC)],
                                start=False, stop=(k == KT - 1),
                            )
                    r_sb = ew_pool.tile([128, JB], BF16, tag="r")
                    nc.scalar.activation(r_sb[:, :], ps_r[:, :], AF.Sigmoid)
                    # n gates (m = 16..23)
                    for j in range(KT):
                        for k in range(KT):
                            m = 2 * KT + j
                            nc.tensor.matmul(
                                ps_n[:, ds(j * BC, BC)],
                                whh_sb[:, k * G3S + m * 128 : k * G3S + (m + 1) * 128],
                                h_cur[:, ds(k * BC, BC)],
                                start=False, stop=(k == KT - 1),
                            )
                    # n chain — pinned early so the scheduler overlaps it
                    # with the z matmul group instead of serializing after
                    if True:
                        rhn = ew_pool.tile([128, JB], F32, tag="rhn")
                        nc.vector.tensor_tensor(rhn[:, :], r_sb[:, :], ps_n[:, :], OP.mult)
                        npre = ew_pool.tile([128, JB], F32, tag="npre")
                        nc.vector.tensor_tensor(
                            npre[:, :], rhn[:, :], xgb[:, ds(xo + 2 * JB, JB)], OP.add)
                        n_sb = ew_pool.tile([128, JB], BF16, tag="n")
                        nc.scalar.activation(n_sb[:, :], npre[:, :], AF.Tanh)
                        d_sb = ew_pool.tile([128, JB], BF16, tag="d")
                        nc.vector.tensor_tensor(d_sb[:, :], h_cur[:, :], n_sb[:, :], OP.subtract)
                    # z gates (m = 8..15)
                    for j in range(KT):
                        for k in range(KT):
                            m = KT + j
                            nc.tensor.matmul(
                                ps_z[:, ds(j * BC, BC)],
                                whh_sb[:, k * G3S + m * 128 : k * G3S + (m + 1) * 128],
                                h_cur[:, ds(k * BC, BC)],
                                start=False, stop=(k == KT - 1),
                            )
                    # tail: sig(z), z*d, h' = n + z*d
                    z_sb = ew_pool.tile([128, JB], BF16, tag="z")
                    nc.scalar.activation(z_sb[:, :], ps_z[:, :], AF.Sigmoid)
                    zd = ew_pool.tile([128, JB], BF16, tag="zd")
                    nc.vector.tensor_tensor(zd[:, :], z_sb[:, :], d_sb[:, :], OP.mult)
                    nc.vector.tensor_tensor(h_nxt[:, :], n_sb[:, :], zd[:, :], OP.add)
                    # interleaved x-projection groups (fill this step's tail)
                    m_lo = (gstep * MT) // (2 * TB)
                    m_hi = ((gstep + 1) * MT) // (2 * TB)
                    for m in range(m_lo, m_hi):
                        psp = p1ps_pool.tile([128, P1N], F32)
                        _p1_group(nc, psp, wih_sb, xt2, m)
                        _p1_evict(nc, stg, psp, biasm_sb, m)

                nc.sync.dma_start(
                    out=xg2[ds(blk + 2, 2), :, :].rearrange("q p f -> p q f"),
                    in_=stg[:, :].rearrange("p (q f) -> p q f", q=2),
                )

        hT_sb = nc.alloc_sbuf_tensor("hT_sb", [128, JB], F32)
        nc.vector.tensor_copy(hT_sb[:, :], h_a[:, :])
        nc.sync.dma_start(out=hT_out[:, :], in_=hT_sb[:, :])

    nc.compile()
    return nc


def _prep_inputs(inputs):
    batch = np.asarray(inputs["batch"], np.float32)
    W_ih = np.asarray(inputs["W_ih"], np.float32)
    W_hh = np.asarray(inputs["W_hh"], np.float32)
    b_ih = np.asarray(inputs["b_ih"], np.float32)
    b_hh = np.asarray(inputs["b_hh"], np.float32)

    wihT = np.ascontiguousarray(W_ih.T).astype(ml_dtypes.bfloat16)
    whhT = np.ascontiguousarray(W_hh.T).astype(ml_dtypes.bfloat16)
    bias = b_ih.copy()
    bias[: 2 * S] += b_hh[: 2 * S]
    biasm = np.ascontiguousarray(bias.reshape(MT, 128).T).astype(np.float32)
    bn = b_hh[2 * S :].reshape(KT, 128).T
    bhhn = np.repeat(bn[:, :, None], BC, axis=2).reshape(128, JB)
    bhhn = bhhn.astype(ml_dtypes.bfloat16)
    identm = np.eye(128, dtype=ml_dtypes.bfloat16)

    in_maps = []
    for c in range(N_CORES):
        bs = batch[c * BC : (c + 1) * BC]
        xTc = np.ascontiguousarray(bs.transpose(2, 1, 0).reshape(S, T * BC))
        xTc = np.concatenate(
            [xTc, np.zeros((S, 2 * TB * BC), xTc.dtype)], axis=1)
        in_maps.append({
            "xT": xTc.astype(ml_dtypes.bfloat16),
            "wih": wihT,
            "whh": whhT,
            "biasm": biasm,
            "bhhn": bhhn,
            "ident": identm,
        })
    return in_maps


class _Runner:
    """Compile once, keep the PJRT executable; run per-core in_maps SPMD."""

    def __init__(self, nc, n_cores):
        bass2jax.install_neuronx_cc_hook()
        self.nc, self.n_cores = nc, n_cores
        pname = nc.partition_id_tensor.name if nc.partition_id_tensor else None
        in_names, out_names, out_avals = [], [], []
        for alloc in nc.m.functions[0].allocations:
            if not isinstance(alloc, mybir.MemoryLocationSet):
                continue
            name = alloc.memorylocations[0].name
            if alloc.kind == "ExternalInput":
                if name != pname:
                    in_names.append(name)
            elif alloc.kind == "ExternalOutput":
                out_names.append(name)
                out_avals.append(
                    jax.core.ShapedArray(tuple(alloc.tensor_shape), mybir.dt.np(alloc.dtype))
                )
        self.in_names, self.out_names, self.out_avals = in_names, out_names, out_avals
        n_params = len(in_names)
        all_in = list(in_names) + list(out_names)
        if pname is not None:
            all_in.append(pname)

        def _body(*args):
            operands = list(args)
            if pname is not None:
                operands.append(bass2jax.partition_id_tensor())
            return tuple(
                bass2jax._bass_exec_p.bind(
                    *operands,
                    out_avals=tuple(out_avals),
                    in_names=tuple(all_in),
                    out_names=tuple(out_names),
                    lowering_input_output_aliases=(),
                    sim_require_finite=False,
                    sim_require_nnan=False,
                    nc=nc,
                )
            )

        devices = jax.devices()[:n_cores]
        self.mesh = Mesh(np.asarray(devices), ("core",))
        specs = (PartitionSpec("core"),) * (n_params + len(out_names))
        self.fn = jax.jit(
            shard_map(_body, mesh=self.mesh, in_specs=specs,
                      out_specs=(PartitionSpec("core"),) * len(out_names),
                      check_rep=False),
            keep_unused=True,
        )

    def run(self, in_maps):
        concat = [
            np.concatenate([np.asarray(in_maps[c][n]) for c in range(self.n_cores)], axis=0)
            for n in self.in_names
        ]
        zeros = [
            np.zeros((self.n_cores * a.shape[0], *a.shape[1:]), a.dtype)
            for a in self.out_avals
        ]
        sh = NamedSharding(self.mesh, PartitionSpec("core"))
        dev = [jax.device_put(a, sh) for a in concat + zeros]
        outs = self.fn(*dev)
        jax.block_until_ready(outs)
        return [
            {
                n: np.asarray(outs[i]).reshape(self.n_cores, *self.out_avals[i].shape)[c]
                for i, n in enumerate(self.out_names)
            }
            for c in range(self.n_cores)
        ]


_CACHED = {}


def kernel(**inputs) -> np.ndarray:
    if "runner" not in _CACHED:
        _CACHED["nc"] = _build()
        _CACHED["runner"] = _Runner(_CACHED["nc"], N_CORES)
    runner = _CACHED["runner"]
    in_maps = _prep_inputs(inputs)
    results = runner.run(in_maps)

    W_out = np.asarray(inputs["W_out"], np.float32)
    b_out = np.asarray(inputs["b_out"], np.float32)
    outs = []
    for c in range(N_CORES):
        hT = np.asarray(results[c]["hT"], np.float32)
        h = hT.reshape(128, KT, BC).transpose(2, 1, 0).reshape(BC, S)
        logits = h @ W_out.T + b_out
        outs.append(1.0 / (1.0 + np.exp(-logits[:, 0])))
    return np.concatenate(outs, 0).astype(np.float32)
